# revision 49
# baseline (speedup 1.0000x reference)
"""GQA attention (RoPE, causal, per-head q-scale) on 8 TRN2 NeuronCores.

Sharding: 2-way data-parallel over batch x 4-way tensor-parallel over heads.
Core c handles batch b=c//4 and head group g=c%4 (8 q heads, 2 kv heads).
Each core computes qkv-proj -> rope -> causal attention -> partial o_proj
(over its heads' columns of Wo); the host sums the 4 partials per batch.

All scalar factors (rope_mscale, sm_scale, per_head_scale) are folded into
the Wq/Wk rows on the host. Causal masking: fully-masked column blocks are
simply skipped (matmul widths trimmed to the causal extent); diagonal
blocks get -BIG added in PSUM via an identity x lower-triangular-constant
matmul before the exp (exp(-BIG) == 0, so the Vaug ones-columns still
produce exact softmax denominators).

dtypes: matmuls run in bf16 with f32 PSUM accumulation; rope runs in bf16
on the vector engine (2x DVE rate) off a gpsimd-staged copy so the QKV
PSUM bank is released after a single op; softmax normalization reads the
PV sums straight out of PSUM (reciprocal_approx_fast, ~18 bits); exps are
causally trimmed via strided access patterns.  Partial o_proj outputs are
written back in bf16 (summed in f64 on host).

Layouts on device (partition, free):
  xt      [hid, s]        hidden^T, streamed in 512-col chunks
  wqkv    [hid, 768]      [Wq(8 heads, scaled) | Wk(2 kv, scaled) | Wv].T
  q/k^T   [d*heads, s]    head-major rows; rope applied in this layout
  scores^T[sk, 2, sq]     per (pair, sk-chunk 128, sq-chunk 512) in PSUM
  exp^T   [sk, 2, sq]     SBUF bf16, fed as matmul rhs
  Vaug    [sk, 128]       V rows (0:64) + 64 ones cols; PV matmul output
                          rows 64:128 then hold the softmax denominators
                          already broadcast over 64 partitions
  out^T   [2d, sq]        PSUM accumulator per (head, sq-chunk)
  attn^T  [o(=2 heads), s] normalized bf16, lhsT for o_proj
  out     [s, hid_out]    partial o_proj result (bf16), one per core
"""

import sys, os

for _p in ("/opt/trn_rl_repo", "/root/.axon_site/_ro/trn_rl_repo"):
    if os.path.isdir(_p) and _p not in sys.path:
        sys.path.insert(0, _p)

import numpy as np

import concourse.bass as bass
import concourse.mybir as mybir
import concourse.tile as tile
from concourse import bacc
from concourse.bass_utils import run_bass_kernel_spmd

F32 = mybir.dt.float32
BF16 = mybir.dt.bfloat16
AF = mybir.ActivationFunctionType

B, S, HID = 2, 2048, 2048
H, K, D = 32, 8, 64
G = H // K
ROPE_MSCALE = 1.2
SM_SCALE = 1.0 / (D ** 0.5)
BIG = 30000.0

NH = 8           # q heads per core
NKV = 2          # kv heads per core
NPAIR = 4        # q head pairs per core
QO = NH * D      # 512 q rows
NK = HID // 128  # 16 contraction chunks
SQW = 512        # sq / xt chunk width
NJ = S // SQW    # 4 chunks
NSK = S // 128   # 16 sk chunks
NWARM = 44       # PE warmup matmuls (cover initial DMA + clock ramp)

_CACHED = {}


def _build():
    if "nc" in _CACHED:
        return _CACHED["nc"]

    nc = bacc.Bacc(None)

    xt_d = nc.declare_dram_parameter("xt", [HID, S], BF16, isOutput=False)
    wqkv_d = nc.declare_dram_parameter("wqkv", [HID, 768], BF16, isOutput=False)
    wo_d = nc.declare_dram_parameter("wo", [QO, HID], BF16, isOutput=False)
    costb_d = nc.declare_dram_parameter("costb", [128, S], BF16, isOutput=False)
    # sin table pre-shifted by +-32 partitions (rows r hold the sin factor
    # that multiplies q row r in the swap-mul whose OUTPUT lands 32 rows
    # away) so both DVE inputs share a base partition (SBUF-SBUF rule).
    sintsb_d = nc.declare_dram_parameter("sintsb", [128, S], BF16, isOutput=False)
    constsb_d = nc.declare_dram_parameter("constsb", [128, 260], BF16, isOutput=False)
    identf_d = nc.declare_dram_parameter("identf", [128, 64], F32, isOutput=False)
    out_d = nc.declare_dram_parameter("out", [S, HID], BF16, isOutput=True)

    with tile.TileContext(nc) as tc:
        # ---------- long-lived pools ----------
        with (
            tc.tile_pool(name="consts", bufs=1) as consts_pool,
            tc.tile_pool(name="ktv", bufs=1) as ktv_pool,
            tc.tile_pool(name="qrope", bufs=10) as qrope_pool,
            tc.tile_pool(name="expt", bufs=6) as expt_pool,
            tc.tile_pool(name="attnt", bufs=8) as attnt_pool,
            tc.tile_pool(name="inv", bufs=2) as inv_pool,
            tc.tile_pool(name="wo", bufs=1) as wo_pool,
            tc.tile_pool(name="ost", bufs=6) as ost_pool,
            tc.tile_pool(name="wq", bufs=1) as wq_pool,
            tc.tile_pool(name="xt", bufs=8) as xt_pool,
            tc.tile_pool(name="cs", bufs=1) as cs_pool,
            tc.tile_pool(name="rtmp", bufs=3) as rtmp_pool,
            tc.tile_pool(name="psc", bufs=2, space="PSUM") as psc_pool,
            tc.tile_pool(name="pout2", bufs=2, space="PSUM") as pout2_pool,
            tc.tile_pool(name="pqkv", bufs=2, space="PSUM") as pqkv_pool,
        ):
            # prioritized loads: first xt chunk + weights feed the first
            # matmuls.  Small tables go out on the (otherwise idle) gpsimd
            # queue so they land within ~3us -- the SP queue issues one DMA
            # every ~0.75us and 20+ deep serial issue starved the tables,
            # head-of-line-blocking the in-order DVE behind their waiters.
            xt_r = xt_d.rearrange("(kc p) s -> p kc s", p=128)
            xt_tiles = {}

            def load_xt(j, eng):
                t = []
                for kq in range(4):
                    tt = xt_pool.tile([128, 4, SQW], BF16, tag="xt", name="xtt")
                    eng.dma_start(
                        out=tt,
                        in_=xt_r[:, kq * 4:(kq + 1) * 4, j * SQW:(j + 1) * SQW],
                    )
                    t.append(tt)
                xt_tiles[j] = t

            # HAM warm-up: dummy matmuls with no DMA deps keep the PE busy
            # during the initial weight/activation loads so the clock gate is
            # already at 8/8 when real work arrives.
            dummy = cs_pool.tile([128, 512], BF16, tag="dummy", name="dummy")
            nc.vector.memset(dummy, 1.0)
            pwarm = pout2_pool.tile([128, 512], F32, tag="p2", name="pwarm")
            for _ in range(NWARM):
                nc.tensor.matmul(
                    pwarm, dummy[:, 0:128], dummy, start=True, stop=True,
                    skip_group_check=True,
                )
            constsb = consts_pool.tile([128, 260], BF16, name="constsb")
            nc.gpsimd.dma_start(out=constsb, in_=constsb_d[:, :])
            identf = consts_pool.tile([128, 64], F32, name="identf")
            nc.gpsimd.dma_start(out=identf, in_=identf_d[:, :])
            costb = cs_pool.tile([128, S], BF16, tag="cost", name="costb")
            sintsb = cs_pool.tile([128, S], BF16, tag="sints", name="sintsb")
            nc.gpsimd.dma_start(out=costb, in_=costb_d[:, :])
            nc.gpsimd.dma_start(out=sintsb, in_=sintsb_d[:, :])
            load_xt(0, nc.sync)
            wqt = wq_pool.tile([128, NK, 768], BF16, name="wqt")
            for k in range(NK):
                nc.sync.dma_start(
                    out=wqt[:, k, :], in_=wqkv_d[k * 128:(k + 1) * 128, :]
                )
            tri01 = constsb[:, 0:128]   # upper-tri (incl diag) ones mask
            ones_colb = constsb[:, 256:257]

            kt_aa = ktv_pool.tile([128, S], BF16, tag="ktaa", name="ktaa")
            kt_bb = ktv_pool.tile([128, S], BF16, tag="ktbb", name="ktbb")
            # Vaug: cols 0:64 = V, cols 64:128 = 1.0 (sums -> rows 64:128 of PV out)
            vaug = [
                ktv_pool.tile([128, NSK, 128], BF16, tag=f"vaug{i}", name=f"vaug{i}")
                for i in range(NKV)
            ]
            for i in range(NKV):
                nc.gpsimd.tensor_copy(
                    vaug[i][:, :, 64:128],
                    ones_colb[:, None, :].broadcast_to([128, NSK, 64]),
                )
            # Wo needed first at oproj(0) during the j=1 window: load last.
            wot = wo_pool.tile([128, NPAIR, HID], BF16, name="wot")
            nc.sync.dma_start(
                out=wot, in_=wo_d.rearrange("(m p) h -> p m h", p=128)
            )

            qrope = {}   # (m, j) -> tile [128, SQW] bf16
            attnt = {}   # (m, j) -> tile [128, SQW] bf16

            def rope(psum_q, j, dst):
                """RoPE a [128, SQW] projected chunk (2 heads) into dst
                (SBUF bf16).  q' = q*cos + swap_halves(q)*sin_signed.
                A single gpsimd copy stages PSUM -> bf16 SBUF (releasing the
                QKV PSUM bank immediately); the swap-multiplies and the
                cos-mul/add then run as bf16 DVE ops at 2x rate."""
                c0, c1 = j * SQW, (j + 1) * SQW
                qs = rtmp_pool.tile([128, SQW], BF16, tag="qs", name="qs")
                nc.scalar.copy(qs, psum_q)
                t2 = rtmp_pool.tile([128, SQW], BF16, tag="t2", name="t2")
                for base in (0, 64):
                    nc.vector.tensor_mul(
                        t2[base:base + 32, :], qs[base + 32:base + 64, :],
                        sintsb[base + 32:base + 64, c0:c1],
                    )
                    nc.vector.tensor_mul(
                        t2[base + 32:base + 64, :], qs[base:base + 32, :],
                        sintsb[base:base + 32, c0:c1],
                    )
                t4 = rtmp_pool.tile([128, SQW], BF16, tag="t4", name="t4")
                nc.vector.tensor_mul(t4, qs, costb[:, c0:c1])
                nc.vector.tensor_add(dst, t2, t4)

            _done_pairs = set()

            def attention_pair(j, m):
                if (j, m) in _done_pairs:
                    return
                _done_pairs.add((j, m))
                nsk = 4 * (j + 1)
                if True:
                    kt = kt_aa if m < 2 else kt_bb
                    va = vaug[m // 2]
                    qr = qrope.pop((m, j))
                    p2 = {}
                    for hb in (0, 64):  # head A at 0, head B at 64
                        p2[hb] = pout2_pool.tile(
                            [128, SQW], F32, tag="p2", name="p2"
                        )
                    pend = []  # staged (exp tile, sk)
                    for sk in range(nsk):
                        # both heads' scores in one 2-bank [128, 2, SQW] tile;
                        # band blocks only compute the causally-valid columns
                        p1 = psc_pool.tile([128, 2, SQW], F32, tag="sc", name="sc")
                        band = sk >= 4 * j
                        off = (sk - 4 * j) * 128 if band else 0
                        for hi, hb in enumerate((0, 64)):
                            nc.tensor.matmul(
                                p1[:, hi, off:SQW],
                                kt[hb:hb + 64, sk * 128:(sk + 1) * 128],
                                qr[hb:hb + 64, off:SQW],
                                start=True,
                                stop=True,
                                skip_group_check=True,
                            )
                        et = expt_pool.tile(
                            [128, 2, SQW], BF16, tag="et", name="et"
                        )
                        # causally-trimmed exp: one strided instruction over
                        # both heads' valid columns
                        nc.scalar.activation(
                            et[:, :, off:SQW], p1[:, :, off:SQW], AF.Exp
                        )
                        if band:
                            # causal mask inside the diagonal block: zero the
                            # sub-diagonal entries on the (otherwise idle)
                            # gpsimd engine instead of -BIG matmuls on the PE;
                            # one strided op covers both heads
                            nc.vector.tensor_mul(
                                et[:, :, off:off + 128],
                                et[:, :, off:off + 128],
                                tri01[:, None, :].broadcast_to([128, 2, 128]),
                            )
                        pend.append((et, sk))
                        while len(pend) > 4 or (sk == nsk - 1 and pend):
                            et2, psk = pend.pop(0)
                            poff = (psk - 4 * j) * 128 if psk >= 4 * j else 0
                            for hi, hb in enumerate((0, 64)):
                                nc.tensor.matmul(
                                    p2[hb][:, poff:SQW],
                                    va[:, psk, :],
                                    et2[:, hi, poff:SQW],
                                    start=(psk == 0),
                                    stop=(psk == nsk - 1),
                                    skip_group_check=True,
                                )
                    # normalize: attnT = out^T * (1/sums); sums come out of
                    # the PV matmul pre-broadcast in psum rows 64:128
                    # reciprocal_approx_fast reads garbage from PSUM (bit-level
                    # seed trick needs an SBUF operand) -> stage sums via SBUF
                    at = attnt_pool.tile([128, SQW], BF16, tag="at", name="at")
                    for hb in (0, 64):
                        sums = inv_pool.tile([64, SQW], F32, tag="sums", name="sums")
                        nc.vector.tensor_copy(sums, p2[hb][64:128, :])
                        invb = inv_pool.tile([64, SQW], F32, tag="invb", name="invb")
                        nc.vector.reciprocal_approx_fast(out=invb, in_=sums)
                        nc.vector.tensor_mul(
                            at[hb:hb + 64, :], p2[hb][0:64, :], invb
                        )
                    attnt[(m, j)] = at

            def oproj_sc(j, sc, tail=False):
                for hc in range(HID // 512):
                    po = pqkv_pool.tile([128, 512], F32, tag="qkv", name="po")
                    for m in range(NPAIR):
                        nc.tensor.matmul(
                            po,
                            attnt[(m, j)][:, sc * 128:(sc + 1) * 128],
                            wot[:, m, hc * 512:(hc + 1) * 512],
                            start=(m == 0),
                            stop=(m == NPAIR - 1),
                        )
                    ot = ost_pool.tile([128, 512], BF16, tag="ot", name="ot")
                    # mid-run: keep the scalar queue free for exps (the next
                    # pair's first scores wait on the psc slot's exp, which
                    # sits behind any scalar copies); in the exp-free tail,
                    # alternate engines so the PSUM ring cycles faster
                    if tail and hc % 2 == 1:
                        nc.scalar.copy(ot, po)
                    else:
                        nc.vector.tensor_copy(ot, po)
                    r0 = j * SQW + sc * 128
                    nc.sync.dma_start(
                        out=out_d[r0:r0 + 128, hc * 512:(hc + 1) * 512],
                        in_=ot,
                    )
                if sc == SQW // 128 - 1:
                    for m in range(NPAIR):
                        attnt.pop((m, j))

            def qkv_chunk(j, m, xt_t):
                pq = pqkv_pool.tile([128, SQW], F32, tag="qkv", name="pqkv")
                for k in range(NK):
                    nc.tensor.matmul(
                        pq,
                        wqt[:, k, m * 128:(m + 1) * 128],
                        xt_t[k // 4][:, k % 4, :],
                        start=(k == 0),
                        stop=(k == NK - 1),
                    )
                if m < NPAIR:
                    qrope[(m, j)] = qrope_pool.tile(
                        [128, SQW], BF16, tag="qr", name="qr"
                    )
                    rope(pq, j, qrope[(m, j)])
                elif m == 4:
                    kro = rtmp_pool.tile([128, SQW], BF16, tag="kro", name="kro")
                    rope(pq, j, kro)
                    c0, c1 = j * SQW, (j + 1) * SQW
                    for dst_b in (0, 64):
                        nc.vector.tensor_copy(
                            kt_aa[dst_b:dst_b + 64, c0:c1], kro[0:64, :]
                        )
                        nc.vector.tensor_copy(
                            kt_bb[dst_b:dst_b + 64, c0:c1], kro[64:128, :]
                        )
                else:
                    # V projected K-style ([2kv*64d rows, s]: one 512-wide
                    # stream of 16 matmuls instead of 4x16 128-wide ones),
                    # then flipped to [s, d] by f32 PE transposes, all landing
                    # in ONE bank-aligned psum tile (single ring slot), fanned
                    # out to vaug with two strided copies.
                    vs = rtmp_pool.tile([128, SQW], F32, tag="vs", name="vs")
                    nc.vector.tensor_copy(vs, pq)
                    sk0 = (j * SQW) // 128
                    for i in range(NKV):
                        hb = i * 64
                        ptv = pqkv_pool.tile([128, 4, 64], F32, tag="qkv", name="ptv")
                        for h2 in range(SQW // 128):
                            nc.tensor.transpose(
                                ptv[:, h2, :],
                                vs[hb:hb + 64, h2 * 128:(h2 + 1) * 128],
                                identf[hb:hb + 64, :],
                            )
                        nc.vector.tensor_copy(
                            vaug[i][:, sk0:sk0 + 4, 0:64], ptv
                        )

            # j=0 projection up front (K first: attention(0,0) needs kt, and
            # the k-rope must not queue behind four q-ropes on the DVE), then
            # software-pipelined emission: attention(j) head-pairs interleaved
            # with QKV(j+1) chunks and o_proj(j-1) row blocks so the PE always
            # has independent work.
            for m in (4, 5, 0, 1, 2, 3):
                qkv_chunk(0, m, xt_tiles[0])
            for j in range(NJ):
                xt_tiles.pop(j)
                if j + 1 < NJ:
                    load_xt(j + 1, nc.sync)
                for m, mq in zip(range(NPAIR), (4, 5, 0, 1)):
                    attention_pair(j, m)
                    if j + 1 < NJ:
                        qkv_chunk(j + 1, mq, xt_tiles[j + 1])
                    if j > 0:
                        oproj_sc(j - 1, m)
                if j + 1 < NJ:
                    qkv_chunk(j + 1, 2, xt_tiles[j + 1])
                    qkv_chunk(j + 1, 3, xt_tiles[j + 1])
                # pull exp-heavy later pairs into earlier, PE-rich windows so
                # the scalar engine (softmax exps) never starves the PE late
                if j == 0:
                    attention_pair(1, 0)
                if j == 1:
                    attention_pair(2, 0)
                if j == NJ - 2:
                    attention_pair(NJ - 1, 0)
                    attention_pair(NJ - 1, 1)
                    attention_pair(NJ - 1, 2)
            for sc in range(4):
                oproj_sc(NJ - 1, sc, tail=True)

    nc.finalize()
    _CACHED["nc"] = nc
    return nc


def _prep_inputs(cos, sin, hidden_states, per_head_scale, Wqkv, Wo):
    """Build the 8 per-core input maps (host-side, free)."""
    import ml_dtypes
    cos = np.asarray(cos, np.float32)
    sin = np.asarray(sin, np.float32)
    hs = np.asarray(hidden_states, np.float32)
    phs = np.asarray(per_head_scale, np.float32)
    Wqkv = np.asarray(Wqkv, np.float32)
    Wo = np.asarray(Wo, np.float32)

    costb = np.ascontiguousarray(np.vstack([cos.T, cos.T])).astype(ml_dtypes.bfloat16)
    # pre-shifted signed sin table (see kernel): row r holds the factor for
    # q row r whose product lands 32 rows away; rows 0:32 = +sin.T[32:64],
    # rows 32:64 = -sin.T[0:32], repeated for the second head.
    st = np.vstack([sin.T[32:64], -sin.T[0:32], sin.T[32:64], -sin.T[0:32]])
    sintsb = np.ascontiguousarray(st).astype(ml_dtypes.bfloat16)

    tri01 = np.triu(np.ones((128, 128), np.float32))  # mask[r,c]=1 iff c>=r
    pad128 = np.zeros((128, 128), np.float32)
    ones_col = np.ones((128, 1), np.float32)
    pad = np.zeros((128, 3), np.float32)
    constsb = np.ascontiguousarray(
        np.concatenate([tri01, pad128, ones_col, pad], axis=1)
    ).astype(ml_dtypes.bfloat16)
    identf = np.ascontiguousarray(
        np.vstack([np.eye(64, dtype=np.float32)] * 2)  # I64 at bases 0 and 64
    )

    xt_b = [np.ascontiguousarray(hs[b].T).astype(ml_dtypes.bfloat16) for b in range(B)]

    in_maps = []
    for c in range(8):
        b, g = c // 4, c % 4
        hq0 = NH * g
        wq = Wqkv[hq0 * D:(hq0 + NH) * D, :].copy()
        for h in range(NH):
            wq[h * D:(h + 1) * D] *= (
                ROPE_MSCALE * SM_SCALE * phs[b, hq0 + h]
            )
        kv0 = H * D + NKV * g * D
        wk = Wqkv[kv0:kv0 + NKV * D, :] * ROPE_MSCALE
        v0 = (H + K) * D + NKV * g * D
        wv = Wqkv[v0:v0 + NKV * D, :]
        wqkv_c = np.ascontiguousarray(np.concatenate([wq, wk, wv], axis=0).T).astype(ml_dtypes.bfloat16)
        in_maps.append({
            "xt": xt_b[b],
            "wqkv": wqkv_c,
            "wo": np.ascontiguousarray(
                Wo[:, hq0 * D:(hq0 + NH) * D].T
            ).astype(ml_dtypes.bfloat16),
            "costb": costb,
            "sintsb": sintsb,
            "constsb": constsb,
            "identf": identf,
        })
    return in_maps


def kernel(cos, sin, hidden_states, per_head_scale, Wqkv, Wo, _trace=False):
    nc = _build()
    in_maps = _prep_inputs(cos, sin, hidden_states, per_head_scale, Wqkv, Wo)
    res = run_bass_kernel_spmd(nc, in_maps, core_ids=list(range(8)), trace=_trace)
    _CACHED["last_results"] = res
    out = np.stack([
        sum(res.results[b * 4 + g]["out"].astype(np.float64) for g in range(4))
        for b in range(B)
    ]).astype(np.float32)
    return out
